# revision 46
# baseline (speedup 1.0000x reference)
"""Trainium2 Bass kernel for nn_AdvancedKANLayer.

Math (reference):
    xn = tanh(x)                                   # [B, I]
    basis[b,i,j,g] = exp(-2*(xn[b,i] - knot[i,j,g])^2)
    spline[b,i,j]  = sum_g basis[b,i,j,g] * coeffs[i,j,g]
    out[b,j]       = sum_i spline[b,i,j] * scale[i,j] + bias[j]

Fast path (knot_positions identical across (i,j), which is how the
reference generates them): basis depends only on (b,i,g), so

    out[b,j] = sum_{i,g} exp(-2*(xn[b,i]-k[g])^2) * (coeffs[i,j,g]*scale[i,j])
             + bias[j]
             = basis2d[b, k] @ W[k, j] + bias[j],   k = g*64 + i  (512 values)

which is a tiny matmul per core after a tanh/square/exp chain.

Sharding: data-parallel over batch. Each of the 8 cores gets B/8 = 256 rows
of x and a replicated copy of the (tiny) parameter tensors. No collectives.

When scale is all-ones / bias all-zeros (runtime-checked; true for this
problem's generator), W equals coeffs up to a pure layout permutation,
which the host applies before the DMA; otherwise W = coeffs*scale and the
+bias are computed on device.

General path (arbitrary knots) evaluates all B*I*J*G basis values.
"""

import numpy as np

B, I, J, G = 2048, 64, 64, 8
NCORES = 8
BS = B // NCORES  # 256 batch rows per core

_cache = {}

# PE fp32 matmul runs each matmul as 2 half-speed passes; float32r is a
# single pass with reduced precision (~2e-4 rel err measured). Keep f32.
USE_F32R = False


def _build_fast(fuse_scale, zero_bias):
    """Bass module for the uniform-knot fast path. Per-core shapes.

    fuse_scale: scale==1 so W chunks arrive pre-arranged via DMA.
    zero_bias:  bias==0 so the final +bias becomes a plain copy.
    """
    import concourse.bass as bass
    import concourse.bacc as bacc
    import concourse.mybir as mybir
    from concourse.tile import TileContext

    f32 = mybir.dt.float32
    mm_dt = mybir.dt.float32r if USE_F32R else f32
    AF = mybir.ActivationFunctionType

    nc = bacc.Bacc(num_devices=NCORES)
    x_h = nc.dram_tensor("x", [BS, I], f32, kind="ExternalInput")
    knots_h = nc.dram_tensor("knots", [G], f32, kind="ExternalInput")
    ident_h = nc.dram_tensor("ident", [128, 128], f32, kind="ExternalInput")
    if fuse_scale:
        # host-permuted coeffs: wmat[c, p, j] = coeffs[i=p%64, j, g=2c+p//64]
        wmat_h = nc.dram_tensor("wmat", [4, 128, J], f32, kind="ExternalInput")
    else:
        coeffs_h = nc.dram_tensor("coeffs", [I, J * G], f32, kind="ExternalInput")
        scale_h = nc.dram_tensor("scale", [I, J], f32, kind="ExternalInput")
    if not zero_bias:
        bias_h = nc.dram_tensor("bias", [J], f32, kind="ExternalInput")
    out_h = nc.dram_tensor("out", [BS, J], f32, kind="ExternalOutput")

    NB = BS // 128  # b-blocks of 128

    with TileContext(nc) as tc:
        with (
            tc.tile_pool(name="consts", bufs=1) as consts,
            tc.tile_pool(name="work", bufs=1) as work,
            tc.tile_pool(name="psum", bufs=1, space="PSUM") as psum_pool,
        ):
            # ---- loads, spread across queues; x first (critical path) ----
            x_sb = work.tile([128, NB, I], f32)
            nc.sync.dma_start(
                out=x_sb[:], in_=x_h[:, :].rearrange("(n p) i -> p n i", p=128)
            )
            identity = consts.tile([128, 128], f32)
            nc.scalar.dma_start(out=identity[:], in_=ident_h[:, :])

            # knots broadcast to all 128 partitions: [128, 8]
            ktile = consts.tile([128, G], f32)
            kap = knots_h[:]
            nc.gpsimd.dma_start(
                out=ktile[:],
                in_=bass.AP(
                    tensor=kap.tensor, offset=kap.offset, ap=[[0, 128], kap.ap[0]]
                ),
            )

            if fuse_scale:
                wmat_sb = consts.tile([128, 4, J], mm_dt)
                nc.sync.dma_start(
                    out=wmat_sb[:],
                    in_=wmat_h[:, :, :].rearrange("c p j -> p c j"),
                )
                w_chunks = [wmat_sb[:, c, :] for c in range(4)]
            else:
                coeffs_sb = consts.tile([I, J * G], f32)
                nc.sync.dma_start(out=coeffs_sb[:], in_=coeffs_h[:, :])
                scale_sb = consts.tile([I, J], f32)
                nc.scalar.dma_start(out=scale_sb[:], in_=scale_h[:, :])

            if not zero_bias:
                bias_bc = consts.tile([128, J], f32)
                bap = bias_h[:]
                nc.gpsimd.dma_start(
                    out=bias_bc[:],
                    in_=bass.AP(
                        tensor=bap.tensor, offset=bap.offset, ap=[[0, 128], bap.ap[0]]
                    ),
                )

            # kneg2[p, c] = -knot[2c + (p>=64)]  -> per-partition square shift
            kneg2 = consts.tile([128, G // 2], f32)
            kt3 = ktile[:].rearrange("p (c two) -> p c two", two=2)
            nc.vector.tensor_scalar_mul(kneg2[0:64, :], kt3[0:64, :, 0], -1.0)
            nc.vector.tensor_scalar_mul(kneg2[64:128, :], kt3[64:128, :, 1], -1.0)
            # knot deltas for the incremental d-chain: dd[:, c] = kneg2[:, c+1]-kneg2[:, c]
            kdd = consts.tile([128, G // 2 - 1], f32)
            nc.vector.tensor_tensor(
                out=kdd[:],
                in0=kneg2[:, 1:4],
                in1=kneg2[:, 0:3],
                op=mybir.AluOpType.subtract,
            )

            # ---- tanh first (no transpose dependency), then transpose ----
            # tanh writes xn twice along the free dim (step-0 re-read of x),
            # so one [128,128] transpose per b-block lands the duplicated
            # [2*64, b] layout in PSUM: xnT2[p, b] = xn[b, p%64].
            xn_sb = work.tile([128, NB, 2, I], f32)
            xap = x_sb[:]
            x_dup = bass.AP(
                tensor=xap.tensor,
                offset=xap.offset,
                ap=[xap.ap[0], xap.ap[1], [0, 2], xap.ap[2]],
            )
            nc.scalar.activation(xn_sb[:], x_dup, AF.Tanh)

            xnT2 = psum_pool.tile([128, NB * 128], f32)
            for n in range(NB):
                nc.tensor.transpose(
                    xnT2[:, 128 * n : 128 * (n + 1)],
                    xn_sb[:, n, :, :],
                    identity[:],
                )

            if not fuse_scale:
                # W chunks: Wc[p, j] = coeffs[i=p%64, j, g=2c+p//64]*scale[i,j]
                coeffs3 = coeffs_sb[:].rearrange("i (j g) -> i j g", g=G)
                w_chunks = []
                for c in range(4):
                    wc = work.tile([128, J], mm_dt, tag=f"w{c}")
                    for h in range(2):
                        nc.gpsimd.tensor_tensor(
                            out=wc[64 * h : 64 * (h + 1), :],
                            in0=coeffs3[:, :, 2 * c + h],
                            in1=scale_sb[:],
                            op=mybir.AluOpType.mult,
                        )
                    w_chunks.append(wc[:])

            # ---- basis: chunk0 squared on ACT straight from PSUM (fast
            # start for the PE); chunks 1-3 via the DVE d-chain:
            # d0 = xnT2 + kneg[0] (PSUM read), d_{c+1} = d_c + dd_c on SBUF.
            b_chunks = []
            d_prev = None
            for c in range(4):
                bc = work.tile([128, NB * 128], mm_dt, tag=f"b{c}")
                if c == 0:
                    nc.scalar.activation(
                        bc[:], xnT2[:], AF.Square, bias=kneg2[:, 0:1], scale=1.0
                    )
                    nc.scalar.activation(bc[:], bc[:], AF.Exp, scale=-2.0)
                else:
                    dc = work.tile([128, NB * 128], f32, tag=f"d{c}")
                    if c == 1:
                        d0 = work.tile([128, NB * 128], f32, tag="d0")
                        nc.vector.tensor_scalar_add(d0[:], xnT2[:], kneg2[:, 0:1])
                        d_prev = d0[:]
                    nc.vector.tensor_scalar_add(dc[:], d_prev, kdd[:, c - 1 : c])
                    nc.vector.tensor_tensor(
                        out=bc[:], in0=dc[:], in1=dc[:], op=mybir.AluOpType.mult
                    )
                    d_prev = dc[:]
                    nc.scalar.activation(bc[:], bc[:], AF.Exp, scale=-2.0)
                b_chunks.append(bc)

            # ---- matmuls: psum[b, j] = sum_c basis_c[b,:] @ Wc ----
            psum_os = [
                psum_pool.tile([128, J], f32, name=f"psum_o{n}") for n in range(NB)
            ]
            out_sb = work.tile([128, NB, J], f32)
            for c in range(4):
                for n in range(NB):
                    nc.tensor.matmul(
                        psum_os[n][:],
                        lhsT=b_chunks[c][:, 128 * n : 128 * (n + 1)],
                        rhs=w_chunks[c],
                        start=(c == 0),
                        stop=(c == 3),
                    )
            for n in range(NB):
                if zero_bias:
                    # one copy on ACT, one on DVE so they run in parallel
                    if n % 2 == 0:
                        nc.scalar.copy(out_sb[:, n, :], psum_os[n][:])
                    else:
                        nc.vector.tensor_copy(out_sb[:, n, :], psum_os[n][:])
                else:
                    nc.vector.tensor_tensor(
                        out=out_sb[:, n, :],
                        in0=psum_os[n][:],
                        in1=bias_bc[:],
                        op=mybir.AluOpType.add,
                    )
                # alternate HWDGE queues so the two stores overlap
                dma_eng = nc.sync if n % 2 == 0 else nc.scalar
                dma_eng.dma_start(
                    out=out_h[:, :].rearrange("(n p) j -> p n j", p=128)[:, n, :],
                    in_=out_sb[:, n, :],
                )

    nc.finalize()
    return nc


def _build_general():
    """Arbitrary-knot path. Layout: (j,g) on partitions in 4 chunks of 128,
    batch on the free dim. Per input-dim i: broadcast xn[:, i] across
    partitions via DMA, ACT computes exp(-2*(xn - k)^2) with the knot as a
    fused per-partition bias, DVE applies w = coeffs*scale, gpsimd
    accumulates over i. Selection matmuls then reduce over g, bias is added
    in [j, b] orientation, and a PE transpose restores [b, j].
    """
    import concourse.bass as bass
    import concourse.bacc as bacc
    import concourse.mybir as mybir
    from concourse.tile import TileContext
    from concourse.masks import make_identity

    f32 = mybir.dt.float32
    AF = mybir.ActivationFunctionType
    Alu = mybir.AluOpType

    nc = bacc.Bacc(num_devices=NCORES)
    x_h = nc.dram_tensor("x", [BS, I], f32, kind="ExternalInput")
    knots_h = nc.dram_tensor("knots", [I, J * G], f32, kind="ExternalInput")
    coeffs_h = nc.dram_tensor("coeffs", [I, J * G], f32, kind="ExternalInput")
    scale_h = nc.dram_tensor("scale", [I, J], f32, kind="ExternalInput")
    bias_h = nc.dram_tensor("bias", [J], f32, kind="ExternalInput")
    out_h = nc.dram_tensor("out", [BS, J], f32, kind="ExternalOutput")

    NB = BS // 128

    with TileContext(nc) as tc:
        with (
            tc.tile_pool(name="consts", bufs=1) as consts,
            tc.tile_pool(name="work", bufs=1) as work,
            tc.tile_pool(name="loop", bufs=3) as loop,
            tc.tile_pool(name="psum", bufs=1, space="PSUM") as psum_pool,
        ):
            # ---- loads ----
            x_sb = work.tile([128, NB, I], f32)
            nc.sync.dma_start(
                out=x_sb[:], in_=x_h[:, :].rearrange("(n p) i -> p n i", p=128)
            )
            knots_sb = consts.tile([I, J * G], f32)
            nc.scalar.dma_start(out=knots_sb[:], in_=knots_h[:, :])
            coeffs_sb = consts.tile([I, J * G], f32)
            nc.sync.dma_start(out=coeffs_sb[:], in_=coeffs_h[:, :])
            scale_sb = consts.tile([I, J], f32)
            nc.scalar.dma_start(out=scale_sb[:], in_=scale_h[:, :])
            bias_sb = consts.tile([J, 1], f32)
            bap = bias_h[:]
            nc.gpsimd.dma_start(
                out=bias_sb[:],
                in_=bass.AP(tensor=bap.tensor, offset=bap.offset, ap=[bap.ap[0], [0, 1]]),
            )

            identity = consts.tile([128, 128], f32)
            make_identity(nc, identity[:])

            # w = coeffs * scale (on DVE, per-g strided), then transposed
            w_sb = work.tile([I, J * G], f32)
            w3 = w_sb[:].rearrange("i (j g) -> i j g", g=G)
            coeffs3 = coeffs_sb[:].rearrange("i (j g) -> i j g", g=G)
            for g in range(G):
                nc.vector.tensor_tensor(
                    out=w3[:, :, g],
                    in0=coeffs3[:, :, g],
                    in1=scale_sb[:],
                    op=Alu.mult,
                )
            psum_w = psum_pool.tile([128, 4, I], f32)
            psum_k = psum_pool.tile([128, 4, I], f32)
            wT = consts.tile([128, 4, I], f32)
            knegT = consts.tile([128, 4, I], f32)
            for c in range(4):
                nc.tensor.transpose(
                    psum_w[:, c, :],
                    w_sb[:, 128 * c : 128 * (c + 1)],
                    identity[0:64, 0:64],
                )
                nc.tensor.transpose(
                    psum_k[:, c, :],
                    knots_sb[:, 128 * c : 128 * (c + 1)],
                    identity[0:64, 0:64],
                )
                nc.vector.tensor_copy(wT[:, c, :], psum_w[:, c, :])
                # negate knots during the PSUM->SBUF copy
                nc.scalar.mul(knegT[:, c, :], psum_k[:, c, :], -1.0)

            # selection matrices S_c[p, j] = (j == 16c + p//8)
            s_mats = []
            for c in range(4):
                sc = consts.tile([128, J], f32, name=f"smat{c}")
                nc.gpsimd.memset(sc[:], 1.0)
                nc.gpsimd.affine_select(
                    out=sc[:], in_=sc[:], pattern=[[-8, J]],
                    compare_op=Alu.is_ge, fill=0.0,
                    base=128 * c, channel_multiplier=1,
                )
                nc.gpsimd.affine_select(
                    out=sc[:], in_=sc[:], pattern=[[8, J]],
                    compare_op=Alu.is_ge, fill=0.0,
                    base=7 - 128 * c, channel_multiplier=-1,
                )
                s_mats.append(sc)

            # xnT = tanh(x).T  [I, BS]
            xn_sb = work.tile([128, NB, I], f32)
            nc.scalar.activation(xn_sb[:], x_sb[:], AF.Tanh)
            psum_x = psum_pool.tile([I, NB * 128], f32)
            for n in range(NB):
                nc.tensor.transpose(
                    psum_x[:, 128 * n : 128 * (n + 1)], xn_sb[:, n, :], identity[:]
                )
            xnT = work.tile([I, NB * 128], f32)
            nc.vector.tensor_copy(xnT[:], psum_x[:])
            # bounce to DRAM: DMA partition-broadcast needs a DRAM source
            xnT_dram = nc.dram_tensor("xnT_scratch", [I, NB * 128], f32)
            nc.sync.dma_start(out=xnT_dram[:, :], in_=xnT[:])

            # accumulators per chunk
            accs = [
                work.tile([128, NB * 128], f32, name=f"acc{c}") for c in range(4)
            ]

            for i in range(I):
                xb = loop.tile([128, NB * 128], f32, tag="xb", bufs=4)
                row = xnT_dram[i, :]
                dma_eng = nc.sync if i % 2 == 0 else nc.scalar
                dma_eng.dma_start(
                    out=xb[:],
                    in_=bass.AP(
                        tensor=row.tensor, offset=row.offset,
                        ap=[[0, 128]] + row.ap,
                    ),
                )
                for c in range(4):
                    sq = loop.tile([128, NB * 128], f32, tag=f"sq{c}", bufs=2)
                    nc.scalar.activation(
                        sq[:], xb[:], AF.Square,
                        bias=knegT[:, c, i : i + 1], scale=1.0,
                    )
                    nc.scalar.activation(sq[:], sq[:], AF.Exp, scale=-2.0)
                    wb = loop.tile([128, NB * 128], f32, tag=f"wb{c}", bufs=2)
                    nc.vector.tensor_scalar_mul(wb[:], sq[:], wT[:, c, i : i + 1])
                    if i == 0:
                        nc.gpsimd.tensor_copy(accs[c][:], wb[:])
                    else:
                        nc.gpsimd.tensor_tensor(
                            out=accs[c][:], in0=accs[c][:], in1=wb[:], op=Alu.add
                        )

            # reduce over g: outT[j, b] = sum_c S_c.T @ acc_c, then +bias
            psum_o = psum_pool.tile([J, NB * 128], f32)
            for c in range(4):
                nc.tensor.matmul(
                    psum_o[:],
                    lhsT=s_mats[c][:],
                    rhs=accs[c][:],
                    start=(c == 0),
                    stop=(c == 3),
                )
            outT = work.tile([J, NB * 128], f32)
            nc.scalar.activation(
                outT[:], psum_o[:], AF.Identity, bias=bias_sb[:, 0:1], scale=1.0
            )

            # transpose back to [b, j] and store
            psum_t = psum_pool.tile([128, NB, J], f32)
            out_sb = work.tile([128, NB, J], f32)
            for n in range(NB):
                nc.tensor.transpose(
                    psum_t[:, n, :],
                    outT[:, 128 * n : 128 * (n + 1)],
                    identity[0:64, 0:64],
                )
                if n % 2 == 0:
                    nc.scalar.copy(out_sb[:, n, :], psum_t[:, n, :])
                else:
                    nc.vector.tensor_copy(out_sb[:, n, :], psum_t[:, n, :])
                dma_eng = nc.sync if n % 2 == 0 else nc.scalar
                dma_eng.dma_start(
                    out=out_h[:, :].rearrange("(n p) j -> p n j", p=128)[:, n, :],
                    in_=out_sb[:, n, :],
                )

    nc.finalize()
    return nc


def _general_in_maps(x, coeffs, knots, scale, bias):
    base = {
        "knots": np.ascontiguousarray(knots.reshape(I, J * G)),
        "coeffs": np.ascontiguousarray(coeffs.reshape(I, J * G)),
        "scale": np.ascontiguousarray(scale),
        "bias": np.ascontiguousarray(bias),
    }
    maps = []
    for i in range(NCORES):
        m = dict(base)
        m["x"] = np.ascontiguousarray(x[i * BS : (i + 1) * BS])
        maps.append(m)
    return maps


def _permute_coeffs(coeffs):
    """wmat[c, p, j] = coeffs[i=p%64, j, g=2c+p//64] — layout only."""
    cg = np.transpose(coeffs, (2, 0, 1))  # [G, I, J]
    return np.ascontiguousarray(cg.reshape(4, 2 * I, J))


def _fast_in_maps(x, coeffs, scale, knots1d, bias, fuse_scale, zero_bias):
    base = {
        "knots": np.ascontiguousarray(knots1d),
        "ident": np.eye(128, dtype=np.float32),
    }
    if fuse_scale:
        base["wmat"] = _permute_coeffs(coeffs)
    else:
        base["coeffs"] = np.ascontiguousarray(coeffs.reshape(I, J * G))
        base["scale"] = np.ascontiguousarray(scale)
    if not zero_bias:
        base["bias"] = np.ascontiguousarray(bias)
    maps = []
    for i in range(NCORES):
        m = dict(base)
        m["x"] = np.ascontiguousarray(x[i * BS : (i + 1) * BS])
        maps.append(m)
    return maps


def _run(nc, in_maps, **kwargs):
    from concourse.bass_utils import run_bass_kernel_spmd

    return run_bass_kernel_spmd(nc, in_maps, core_ids=list(range(NCORES)), **kwargs)


def kernel(x, spline_coeffs, knot_positions, scale, bias, _trace=False):
    x = np.asarray(x, dtype=np.float32)
    coeffs = np.asarray(spline_coeffs, dtype=np.float32)
    knots = np.asarray(knot_positions, dtype=np.float32)
    scale = np.asarray(scale, dtype=np.float32)
    bias = np.asarray(bias, dtype=np.float32)

    uniform = bool(np.all(knots == knots[0, 0]))
    if not uniform:
        if "general" not in _cache:
            _cache["general"] = _build_general()
        nc = _cache["general"]
        in_maps = _general_in_maps(x, coeffs, knots, scale, bias)
        res = _run(nc, in_maps, trace=_trace)
        out = np.concatenate(
            [res.results[i]["out"] for i in range(NCORES)], axis=0
        )
        return (out, res) if _trace else out

    fuse_scale = bool(np.all(scale == 1.0))
    zero_bias = bool(np.all(bias == 0.0))
    key = ("fast", fuse_scale, zero_bias)
    if key not in _cache:
        _cache[key] = _build_fast(fuse_scale, zero_bias)
    nc = _cache[key]
    in_maps = _fast_in_maps(x, coeffs, scale, knots[0, 0], bias, fuse_scale, zero_bias)
    res = _run(nc, in_maps, trace=_trace)
    out = np.concatenate([res.results[i]["out"] for i in range(NCORES)], axis=0)
    if _trace:
        return out, res
    return out


# revision 48
# speedup vs baseline: 13.5648x; 13.5648x over previous
"""Trainium2 Bass kernel for nn_AdvancedKANLayer.

Math (reference):
    xn = tanh(x)                                   # [B, I]
    basis[b,i,j,g] = exp(-2*(xn[b,i] - knot[i,j,g])^2)
    spline[b,i,j]  = sum_g basis[b,i,j,g] * coeffs[i,j,g]
    out[b,j]       = sum_i spline[b,i,j] * scale[i,j] + bias[j]

Fast path (knot_positions identical across (i,j), which is how the
reference generates them): basis depends only on (b,i,g), so

    out[b,j] = sum_{i,g} exp(-2*(xn[b,i]-k[g])^2) * (coeffs[i,j,g]*scale[i,j])
             + bias[j]
             = basis2d[b, k] @ W[k, j] + bias[j],   k = g*64 + i  (512 values)

which is a tiny matmul per core after a tanh/square/exp chain.

Sharding: data-parallel over batch. Each of the 8 cores gets B/8 = 256 rows
of x and a replicated copy of the (tiny) parameter tensors. No collectives.

When scale is all-ones / bias all-zeros (runtime-checked; true for this
problem's generator), W equals coeffs up to a pure layout permutation,
which the host applies before the DMA; otherwise W = coeffs*scale and the
+bias are computed on device.

General path (arbitrary knots) evaluates all B*I*J*G basis values.
"""

import numpy as np

B, I, J, G = 2048, 64, 64, 8
NCORES = 8
BS = B // NCORES  # 256 batch rows per core

_cache = {}

# PE fp32 matmul runs each matmul as 2 half-speed passes; float32r is a
# single pass with reduced precision (~2e-4 rel err measured). Keep f32.
USE_F32R = False


def _build_fast(fuse_scale, zero_bias):
    """Bass module for the uniform-knot fast path. Per-core shapes.

    fuse_scale: scale==1 so W chunks arrive pre-arranged via DMA.
    zero_bias:  bias==0 so the final +bias becomes a plain copy.
    """
    import concourse.bass as bass
    import concourse.bacc as bacc
    import concourse.mybir as mybir
    from concourse.tile import TileContext

    f32 = mybir.dt.float32
    mm_dt = mybir.dt.float32r if USE_F32R else f32
    AF = mybir.ActivationFunctionType

    nc = bacc.Bacc(num_devices=NCORES)
    x_h = nc.dram_tensor("x", [BS, I], f32, kind="ExternalInput")
    knots_h = nc.dram_tensor("knots", [G], f32, kind="ExternalInput")
    ident_h = nc.dram_tensor("ident", [128, 128], f32, kind="ExternalInput")
    if fuse_scale:
        # host-permuted coeffs: wmat[c, p, j] = coeffs[i=p%64, j, g=2c+p//64]
        wmat_h = nc.dram_tensor("wmat", [4, 128, J], f32, kind="ExternalInput")
    else:
        coeffs_h = nc.dram_tensor("coeffs", [I, J * G], f32, kind="ExternalInput")
        scale_h = nc.dram_tensor("scale", [I, J], f32, kind="ExternalInput")
    if not zero_bias:
        bias_h = nc.dram_tensor("bias", [J], f32, kind="ExternalInput")
    out_h = nc.dram_tensor("out", [BS, J], f32, kind="ExternalOutput")

    NB = BS // 128  # b-blocks of 128

    with TileContext(nc) as tc:
        with (
            tc.tile_pool(name="consts", bufs=1) as consts,
            tc.tile_pool(name="work", bufs=1) as work,
            tc.tile_pool(name="psum", bufs=1, space="PSUM") as psum_pool,
        ):
            # ---- loads, spread across queues; x first (critical path) ----
            x_sb = work.tile([128, NB, I], f32)
            nc.sync.dma_start(
                out=x_sb[:], in_=x_h[:, :].rearrange("(n p) i -> p n i", p=128)
            )
            identity = consts.tile([128, 128], f32)
            nc.scalar.dma_start(out=identity[:], in_=ident_h[:, :])

            # knots broadcast to all 128 partitions: [128, 8]
            ktile = consts.tile([128, G], f32)
            kap = knots_h[:]
            nc.gpsimd.dma_start(
                out=ktile[:],
                in_=bass.AP(
                    tensor=kap.tensor, offset=kap.offset, ap=[[0, 128], kap.ap[0]]
                ),
            )

            if fuse_scale:
                wmat_sb = consts.tile([128, 4, J], mm_dt)
                nc.sync.dma_start(
                    out=wmat_sb[:],
                    in_=wmat_h[:, :, :].rearrange("c p j -> p c j"),
                )
                w_chunks = [wmat_sb[:, c, :] for c in range(4)]
            else:
                coeffs_sb = consts.tile([I, J * G], f32)
                nc.sync.dma_start(out=coeffs_sb[:], in_=coeffs_h[:, :])
                scale_sb = consts.tile([I, J], f32)
                nc.scalar.dma_start(out=scale_sb[:], in_=scale_h[:, :])

            if not zero_bias:
                bias_bc = consts.tile([128, J], f32)
                bap = bias_h[:]
                nc.gpsimd.dma_start(
                    out=bias_bc[:],
                    in_=bass.AP(
                        tensor=bap.tensor, offset=bap.offset, ap=[[0, 128], bap.ap[0]]
                    ),
                )

            # kneg2[p, c] = -knot[2c + (p>=64)]  -> per-partition square shift
            kneg2 = consts.tile([128, G // 2], f32)
            kt3 = ktile[:].rearrange("p (c two) -> p c two", two=2)
            nc.vector.tensor_scalar_mul(kneg2[0:64, :], kt3[0:64, :, 0], -1.0)
            nc.vector.tensor_scalar_mul(kneg2[64:128, :], kt3[64:128, :, 1], -1.0)
            # knot deltas for the incremental d-chain: dd[:, c] = kneg2[:, c+1]-kneg2[:, c]
            kdd = consts.tile([128, G // 2 - 1], f32)
            nc.vector.tensor_tensor(
                out=kdd[:],
                in0=kneg2[:, 1:4],
                in1=kneg2[:, 0:3],
                op=mybir.AluOpType.subtract,
            )

            # ---- tanh first (no transpose dependency), then transpose ----
            # tanh writes xn twice along the free dim (step-0 re-read of x),
            # so one [128,128] transpose per b-block lands the duplicated
            # [2*64, b] layout in PSUM: xnT2[p, b] = xn[b, p%64].
            xn_sb = work.tile([128, NB, 2, I], f32)
            xap = x_sb[:]
            x_dup = bass.AP(
                tensor=xap.tensor,
                offset=xap.offset,
                ap=[xap.ap[0], xap.ap[1], [0, 2], xap.ap[2]],
            )
            nc.scalar.activation(xn_sb[:], x_dup, AF.Tanh)

            xnT2 = psum_pool.tile([128, NB * 128], f32)
            for n in range(NB):
                nc.tensor.transpose(
                    xnT2[:, 128 * n : 128 * (n + 1)],
                    xn_sb[:, n, :, :],
                    identity[:],
                )

            if not fuse_scale:
                # W chunks: Wc[p, j] = coeffs[i=p%64, j, g=2c+p//64]*scale[i,j]
                coeffs3 = coeffs_sb[:].rearrange("i (j g) -> i j g", g=G)
                w_chunks = []
                for c in range(4):
                    wc = work.tile([128, J], mm_dt, tag=f"w{c}")
                    for h in range(2):
                        nc.gpsimd.tensor_tensor(
                            out=wc[64 * h : 64 * (h + 1), :],
                            in0=coeffs3[:, :, 2 * c + h],
                            in1=scale_sb[:],
                            op=mybir.AluOpType.mult,
                        )
                    w_chunks.append(wc[:])

            # ---- basis: chunk0 squared on ACT straight from PSUM (fast
            # start for the PE); chunks 1-3 via the DVE d-chain:
            # d0 = xnT2 + kneg[0] (PSUM read), d_{c+1} = d_c + dd_c on SBUF.
            b_chunks = []
            d_prev = None
            for c in range(4):
                bc = work.tile([128, NB * 128], mm_dt, tag=f"b{c}")
                if c == 0:
                    nc.scalar.activation(
                        bc[:], xnT2[:], AF.Square, bias=kneg2[:, 0:1], scale=1.0
                    )
                    nc.scalar.activation(bc[:], bc[:], AF.Exp, scale=-2.0)
                else:
                    dc = work.tile([128, NB * 128], f32, tag=f"d{c}")
                    if c == 1:
                        d0 = work.tile([128, NB * 128], f32, tag="d0")
                        nc.vector.tensor_scalar_add(d0[:], xnT2[:], kneg2[:, 0:1])
                        d_prev = d0[:]
                    nc.vector.tensor_scalar_add(dc[:], d_prev, kdd[:, c - 1 : c])
                    nc.vector.tensor_tensor(
                        out=bc[:], in0=dc[:], in1=dc[:], op=mybir.AluOpType.mult
                    )
                    d_prev = dc[:]
                    nc.scalar.activation(bc[:], bc[:], AF.Exp, scale=-2.0)
                b_chunks.append(bc)

            # ---- matmuls: psum[b, j] = sum_c basis_c[b,:] @ Wc ----
            psum_os = [
                psum_pool.tile([128, J], f32, name=f"psum_o{n}") for n in range(NB)
            ]
            out_sb = work.tile([128, NB, J], f32)
            for c in range(4):
                for n in range(NB):
                    nc.tensor.matmul(
                        psum_os[n][:],
                        lhsT=b_chunks[c][:, 128 * n : 128 * (n + 1)],
                        rhs=w_chunks[c],
                        start=(c == 0),
                        stop=(c == 3),
                    )
            for n in range(NB):
                if zero_bias:
                    # one copy on ACT, one on DVE so they run in parallel
                    if n % 2 == 0:
                        nc.scalar.copy(out_sb[:, n, :], psum_os[n][:])
                    else:
                        nc.vector.tensor_copy(out_sb[:, n, :], psum_os[n][:])
                else:
                    nc.vector.tensor_tensor(
                        out=out_sb[:, n, :],
                        in0=psum_os[n][:],
                        in1=bias_bc[:],
                        op=mybir.AluOpType.add,
                    )
                # alternate HWDGE queues so the two stores overlap
                dma_eng = nc.sync if n % 2 == 0 else nc.scalar
                dma_eng.dma_start(
                    out=out_h[:, :].rearrange("(n p) j -> p n j", p=128)[:, n, :],
                    in_=out_sb[:, n, :],
                )

    nc.finalize()
    return nc


def _build_general():
    """Arbitrary-knot path. Layout: (j,g) on partitions in 4 chunks of 128,
    batch on the free dim. Per input-dim i: broadcast xn[:, i] across
    partitions via DMA, ACT computes exp(-2*(xn - k)^2) with the knot as a
    fused per-partition bias, DVE applies w = coeffs*scale, gpsimd
    accumulates over i. Selection matmuls then reduce over g, bias is added
    in [j, b] orientation, and a PE transpose restores [b, j].
    """
    import concourse.bass as bass
    import concourse.bacc as bacc
    import concourse.mybir as mybir
    from concourse.tile import TileContext
    from concourse.masks import make_identity

    f32 = mybir.dt.float32
    AF = mybir.ActivationFunctionType
    Alu = mybir.AluOpType

    nc = bacc.Bacc(num_devices=NCORES)
    x_h = nc.dram_tensor("x", [BS, I], f32, kind="ExternalInput")
    knots_h = nc.dram_tensor("knots", [I, J * G], f32, kind="ExternalInput")
    coeffs_h = nc.dram_tensor("coeffs", [I, J * G], f32, kind="ExternalInput")
    scale_h = nc.dram_tensor("scale", [I, J], f32, kind="ExternalInput")
    bias_h = nc.dram_tensor("bias", [J], f32, kind="ExternalInput")
    out_h = nc.dram_tensor("out", [BS, J], f32, kind="ExternalOutput")

    NB = BS // 128

    with TileContext(nc) as tc:
        with (
            tc.tile_pool(name="consts", bufs=1) as consts,
            tc.tile_pool(name="work", bufs=1) as work,
            tc.tile_pool(name="loop", bufs=3) as loop,
            tc.tile_pool(name="psum", bufs=1, space="PSUM") as psum_pool,
        ):
            # ---- loads ----
            x_sb = work.tile([128, NB, I], f32)
            nc.sync.dma_start(
                out=x_sb[:], in_=x_h[:, :].rearrange("(n p) i -> p n i", p=128)
            )
            knots_sb = consts.tile([I, J * G], f32)
            nc.scalar.dma_start(out=knots_sb[:], in_=knots_h[:, :])
            coeffs_sb = consts.tile([I, J * G], f32)
            nc.sync.dma_start(out=coeffs_sb[:], in_=coeffs_h[:, :])
            scale_sb = consts.tile([I, J], f32)
            nc.scalar.dma_start(out=scale_sb[:], in_=scale_h[:, :])
            bias_sb = consts.tile([J, 1], f32)
            bap = bias_h[:]
            nc.gpsimd.dma_start(
                out=bias_sb[:],
                in_=bass.AP(tensor=bap.tensor, offset=bap.offset, ap=[bap.ap[0], [0, 1]]),
            )

            identity = consts.tile([128, 128], f32)
            make_identity(nc, identity[:])

            # w = coeffs * scale (on DVE, per-g strided), then transposed
            w_sb = work.tile([I, J * G], f32)
            w3 = w_sb[:].rearrange("i (j g) -> i j g", g=G)
            coeffs3 = coeffs_sb[:].rearrange("i (j g) -> i j g", g=G)
            for g in range(G):
                nc.vector.tensor_tensor(
                    out=w3[:, :, g],
                    in0=coeffs3[:, :, g],
                    in1=scale_sb[:],
                    op=Alu.mult,
                )
            psum_w = psum_pool.tile([128, 4, I], f32)
            psum_k = psum_pool.tile([128, 4, I], f32)
            wT = consts.tile([128, 4, I], f32)
            knegT = consts.tile([128, 4, I], f32)
            for c in range(4):
                nc.tensor.transpose(
                    psum_w[:, c, :],
                    w_sb[:, 128 * c : 128 * (c + 1)],
                    identity[0:64, 0:64],
                )
                nc.tensor.transpose(
                    psum_k[:, c, :],
                    knots_sb[:, 128 * c : 128 * (c + 1)],
                    identity[0:64, 0:64],
                )
                nc.vector.tensor_copy(wT[:, c, :], psum_w[:, c, :])
                # negate knots during the PSUM->SBUF copy
                nc.scalar.mul(knegT[:, c, :], psum_k[:, c, :], -1.0)

            # selection matrices S_c[p, j] = (j == 16c + p//8)
            s_mats = []
            for c in range(4):
                sc = consts.tile([128, J], f32, name=f"smat{c}")
                nc.gpsimd.memset(sc[:], 1.0)
                nc.gpsimd.affine_select(
                    out=sc[:], in_=sc[:], pattern=[[-8, J]],
                    compare_op=Alu.is_ge, fill=0.0,
                    base=128 * c, channel_multiplier=1,
                )
                nc.gpsimd.affine_select(
                    out=sc[:], in_=sc[:], pattern=[[8, J]],
                    compare_op=Alu.is_ge, fill=0.0,
                    base=7 - 128 * c, channel_multiplier=-1,
                )
                s_mats.append(sc)

            # xnT = tanh(x).T  [I, BS]
            xn_sb = work.tile([128, NB, I], f32)
            nc.scalar.activation(xn_sb[:], x_sb[:], AF.Tanh)
            psum_x = psum_pool.tile([I, NB * 128], f32)
            for n in range(NB):
                nc.tensor.transpose(
                    psum_x[:, 128 * n : 128 * (n + 1)], xn_sb[:, n, :], identity[:]
                )
            xnT = work.tile([I, NB * 128], f32)
            nc.vector.tensor_copy(xnT[:], psum_x[:])
            # bounce to DRAM: DMA partition-broadcast needs a DRAM source
            xnT_dram = nc.dram_tensor("xnT_scratch", [I, NB * 128], f32)
            nc.sync.dma_start(out=xnT_dram[:, :], in_=xnT[:])

            # accumulators per chunk
            accs = [
                work.tile([128, NB * 128], f32, name=f"acc{c}") for c in range(4)
            ]

            for i in range(I):
                xb = loop.tile([128, NB * 128], f32, tag="xb", bufs=4)
                row = xnT_dram[i, :]
                dma_eng = nc.sync if i % 2 == 0 else nc.scalar
                dma_eng.dma_start(
                    out=xb[:],
                    in_=bass.AP(
                        tensor=row.tensor, offset=row.offset,
                        ap=[[0, 128]] + row.ap,
                    ),
                )
                for c in range(4):
                    sq = loop.tile([128, NB * 128], f32, tag=f"sq{c}", bufs=2)
                    nc.scalar.activation(
                        sq[:], xb[:], AF.Square,
                        bias=knegT[:, c, i : i + 1], scale=1.0,
                    )
                    nc.scalar.activation(sq[:], sq[:], AF.Exp, scale=-2.0)
                    wb = loop.tile([128, NB * 128], f32, tag=f"wb{c}", bufs=2)
                    nc.vector.tensor_scalar_mul(wb[:], sq[:], wT[:, c, i : i + 1])
                    if i == 0:
                        nc.gpsimd.tensor_copy(accs[c][:], wb[:])
                    else:
                        nc.gpsimd.tensor_tensor(
                            out=accs[c][:], in0=accs[c][:], in1=wb[:], op=Alu.add
                        )

            # reduce over g: outT[j, b] = sum_c S_c.T @ acc_c, then +bias
            psum_o = psum_pool.tile([J, NB * 128], f32)
            for c in range(4):
                nc.tensor.matmul(
                    psum_o[:],
                    lhsT=s_mats[c][:],
                    rhs=accs[c][:],
                    start=(c == 0),
                    stop=(c == 3),
                )
            outT = work.tile([J, NB * 128], f32)
            nc.scalar.activation(
                outT[:], psum_o[:], AF.Identity, bias=bias_sb[:, 0:1], scale=1.0
            )

            # transpose back to [b, j] and store
            psum_t = psum_pool.tile([128, NB, J], f32)
            out_sb = work.tile([128, NB, J], f32)
            for n in range(NB):
                nc.tensor.transpose(
                    psum_t[:, n, :],
                    outT[:, 128 * n : 128 * (n + 1)],
                    identity[0:64, 0:64],
                )
                if n % 2 == 0:
                    nc.scalar.copy(out_sb[:, n, :], psum_t[:, n, :])
                else:
                    nc.vector.tensor_copy(out_sb[:, n, :], psum_t[:, n, :])
                dma_eng = nc.sync if n % 2 == 0 else nc.scalar
                dma_eng.dma_start(
                    out=out_h[:, :].rearrange("(n p) j -> p n j", p=128)[:, n, :],
                    in_=out_sb[:, n, :],
                )

    nc.finalize()
    return nc


def _general_in_maps(x, coeffs, knots, scale, bias):
    base = {
        "knots": np.ascontiguousarray(knots.reshape(I, J * G)),
        "coeffs": np.ascontiguousarray(coeffs.reshape(I, J * G)),
        "scale": np.ascontiguousarray(scale),
        "bias": np.ascontiguousarray(bias),
    }
    maps = []
    for i in range(NCORES):
        m = dict(base)
        m["x"] = np.ascontiguousarray(x[i * BS : (i + 1) * BS])
        maps.append(m)
    return maps


def _permute_coeffs(coeffs):
    """wmat[c, p, j] = coeffs[i=p%64, j, g=2c+p//64] — layout only."""
    cg = np.transpose(coeffs, (2, 0, 1))  # [G, I, J]
    return np.ascontiguousarray(cg.reshape(4, 2 * I, J))


def _fast_in_maps(x, coeffs, scale, knots1d, bias, fuse_scale, zero_bias):
    base = {
        "knots": np.ascontiguousarray(knots1d),
        "ident": np.eye(128, dtype=np.float32),
    }
    if fuse_scale:
        base["wmat"] = _permute_coeffs(coeffs)
    else:
        base["coeffs"] = np.ascontiguousarray(coeffs.reshape(I, J * G))
        base["scale"] = np.ascontiguousarray(scale)
    if not zero_bias:
        base["bias"] = np.ascontiguousarray(bias)
    maps = []
    for i in range(NCORES):
        m = dict(base)
        m["x"] = np.ascontiguousarray(x[i * BS : (i + 1) * BS])
        maps.append(m)
    return maps


def _run(nc, in_maps, **kwargs):
    from concourse.bass_utils import run_bass_kernel_spmd

    return run_bass_kernel_spmd(nc, in_maps, core_ids=list(range(NCORES)), **kwargs)


def kernel(x, spline_coeffs, knot_positions, scale, bias, _trace=False):
    x = np.asarray(x, dtype=np.float32)
    coeffs = np.asarray(spline_coeffs, dtype=np.float32)
    knots = np.asarray(knot_positions, dtype=np.float32)
    scale = np.asarray(scale, dtype=np.float32)
    bias = np.asarray(bias, dtype=np.float32)

    uniform = bool(np.all(knots == knots[0, 0]))
    if not uniform:
        if "general" not in _cache:
            _cache["general"] = _build_general()
        nc = _cache["general"]
        in_maps = _general_in_maps(x, coeffs, knots, scale, bias)
        res = _run(nc, in_maps, trace=_trace)
        out = np.concatenate(
            [res.results[i]["out"] for i in range(NCORES)], axis=0
        )
        return (out, res) if _trace else out

    fuse_scale = bool(np.all(scale == 1.0))
    zero_bias = bool(np.all(bias == 0.0))
    key = ("fast", fuse_scale, zero_bias)
    if key not in _cache:
        _cache[key] = _build_fast(fuse_scale, zero_bias)
    nc = _cache[key]
    in_maps = _fast_in_maps(x, coeffs, scale, knots[0, 0], bias, fuse_scale, zero_bias)
    res = _run(nc, in_maps, trace=_trace)
    out = np.concatenate([res.results[i]["out"] for i in range(NCORES)], axis=0)
    if _trace:
        return out, res
    return out
